# revision 42
# baseline (speedup 1.0000x reference)
"""Trainium2 Bass kernel for nn_DCNModel_12816182411985.

Model: DCN — shared deep MLP (1024->500->200->200 with relu) + 2-task
cross-net + sigmoid heads on concat([emb, d3]) @ Wl.

Key algebraic collapse: the cross-net iteration
    emb_{j+1} = s * emb_j * cw[i,j] + cb[i,j] + x      (s = sum(x, axis=1))
is affine per (batch, feature), so emb3 = x * P_i(s) + Q_i(s) with cubic
polynomials in s whose coefficients are per-feature vectors.  Hence

  emb3 @ w_emb = (x@w) + s*(x@(cw2*w)) + s^2*(x@(cw1*cw2*w)) + s^3*(x@(cw0*cw1*cw2*w))
                 + s*(cb1*cw2 . w) + s^2*(cb0*cw1*cw2 . w) + (cb2 . w)

All x-projections (8 columns incl. a ones-column producing s) are folded
into the big x @ W1 matmul as extra output columns.  The per-batch cubic
combine is done with a few DVE row ops + one tiny selection matmul that
also accumulates d3 @ Wl_d3.

Sharding: data-parallel batch split across 8 cores; weights replicated.
Numerics: bf16 weights/activations on the wire and in SBUF, fp32 PSUM
accumulation (halves both the axon-tunnel transfer and HBM traffic; the
tail columns reach 1e-8 magnitudes so fp16 is out of range, bf16 is not).
Orientation: features on partitions, batch on the free axis; x is
transposed on-chip via PE transpose-mode.

Execution: direct jit(shard_map) over the 8 cores — x is passed as one
(16384, 1024) array and split by the mesh (no host-side slice/concat
roundtrip); weights are replicated via P().  A benchmark variant wraps
the whole body in tc.For_i(0, K) so per-iteration HW time can be
measured without the per-dispatch tunnel overhead.
"""

import numpy as np

B, DIM = 16384, 1024
H1, H2, H3 = 500, 200, 200
NCORES = 8
BPC = B // NCORES        # 2048 batch rows per core
NTILE = 512              # batch columns per tile
NT = BPC // NTILE        # 4 column tiles per core
NCH = NTILE // 128       # 4 batch chunks of 128 per column tile
KF = DIM // 128          # 8 feature k-tiles

# d1 row layout (after column permutation of W1):
#   rows   0:480  -> W1 cols 0:480
#   rows 480:488  -> tail block [s, y1_0, y2_0, y3_0, y1_1, y2_1, y3_1, y0]
#   rows 488:508  -> W1 cols 480:500
#   rows 508:512  -> zero pad
# In m-tile 3 (partitions 0..127 <-> rows 384..511) the tail block sits at
# partitions 96..103 (32-aligned, as required for matmul tile_position).

_CACHE = {}
LAST_RESULTS = None  # kept for test.py compatibility (always None now)


def _np_bf16():
    import concourse.mybir as mybir
    return mybir.dt.np(mybir.dt.bfloat16)


def _build_nc(loop=None):
    """Build the Bass program.  loop=None: single pass.  loop=K: the whole
    per-batch body re-executes K times under tc.For_i (same data, same
    output) — used only for timing."""
    import concourse.bacc as bacc
    import concourse.mybir as mybir
    import concourse.tile as tile
    from contextlib import ExitStack, nullcontext

    f32 = mybir.dt.float32
    bf16 = mybir.dt.bfloat16
    AF = mybir.ActivationFunctionType

    nc = bacc.Bacc("TRN2", target_bir_lowering=False, debug=False)

    x_d = nc.dram_tensor("xT_shard", [DIM, BPC], bf16, kind="ExternalInput")
    w1_d = nc.dram_tensor("w1aug", [DIM, 512], bf16, kind="ExternalInput")
    w2_d = nc.dram_tensor("w2aug", [512, H2], bf16, kind="ExternalInput")
    w3_d = nc.dram_tensor("w3m", [H2, H3], bf16, kind="ExternalInput")
    wd3_d = nc.dram_tensor("wd3dup", [H3, 2], bf16, kind="ExternalInput")
    sel_d = nc.dram_tensor("sel", [128, 2], bf16, kind="ExternalInput")
    b1_d = nc.dram_tensor("b1aug", [128, 4], f32, kind="ExternalInput")
    b2_d = nc.dram_tensor("b2arr", [100, 2], f32, kind="ExternalInput")
    b3_d = nc.dram_tensor("b3arr", [100, 2], f32, kind="ExternalInput")
    sigb_d = nc.dram_tensor("sigb", [2, 1], f32, kind="ExternalInput")
    mask_d = nc.dram_tensor("tailmask", [128, 6], f32, kind="ExternalInput")
    ones_d = nc.dram_tensor("onesrow", [1, 8], bf16, kind="ExternalInput")
    out_d = nc.dram_tensor("preds", [2, BPC], f32, kind="ExternalOutput")

    with tile.TileContext(nc) as tc, ExitStack() as stack:
        # ---------- constants / weights (resident for the whole kernel) ----
        consts_pool = stack.enter_context(tc.tile_pool(name="consts", bufs=1))

        def single(shape, name, dtype=f32):
            return consts_pool.tile(shape, dtype, name=name, tag=name)

        w1sb = []
        for f in range(KF):
            t = single([128, 512], f"w1sb{f}", bf16)
            nc.sync.dma_start(out=t, in_=w1_d[f * 128:(f + 1) * 128, :])
            w1sb.append(t)
        w2sb = []
        for k in range(4):
            t = single([128, H2], f"w2sb{k}", bf16)
            nc.sync.dma_start(out=t, in_=w2_d[k * 128:(k + 1) * 128, :])
            w2sb.append(t)
        w3sb = []
        for k in range(2):
            t = single([100, H3], f"w3sb{k}", bf16)
            nc.sync.dma_start(out=t, in_=w3_d[k * 100:(k + 1) * 100, :])
            w3sb.append(t)
        wd3sb = []
        for k in range(2):
            t = single([100, 2], f"wd3sb{k}", bf16)
            nc.sync.dma_start(out=t, in_=wd3_d[k * 100:(k + 1) * 100, :])
            wd3sb.append(t)
        selsb = single([128, 2], "selsb", bf16)
        nc.sync.dma_start(out=selsb, in_=sel_d[:, :])
        b1sb = single([128, 4], "b1sb")
        nc.sync.dma_start(out=b1sb, in_=b1_d[:, :])
        b2sb = single([100, 2], "b2sb")
        nc.sync.dma_start(out=b2sb, in_=b2_d[:, :])
        b3sb = single([100, 2], "b3sb")
        nc.sync.dma_start(out=b3sb, in_=b3_d[:, :])
        sigbsb = single([2, 1], "sigbsb")
        nc.sync.dma_start(out=sigbsb, in_=sigb_d[:, :])
        maskbuf = single([128, 6], "maskbuf")
        nc.sync.dma_start(out=maskbuf, in_=mask_d[:, :])
        ones8 = single([128, 8], "ones8", bf16)
        nc.sync.dma_start(out=ones8[96:97, :], in_=ones_d[0:1, 0:8])

        with (
            tc.tile_pool(name="xT", bufs=16) as xt_pool,
            tc.tile_pool(name="d1p", bufs=20) as d1_pool,
            tc.tile_pool(name="d2p", bufs=3) as d2_pool,
            tc.tile_pool(name="d3p", bufs=3) as d3_pool,
            tc.tile_pool(name="osbp", bufs=2) as out_pool,
            tc.tile_pool(name="tmpp", bufs=2) as tmp_pool,
            tc.tile_pool(name="ssp", bufs=2) as ss_pool,
            tc.tile_pool(name="pl1", bufs=4, space="PSUM") as pl1,
            tc.tile_pool(name="pl2", bufs=1, space="PSUM") as pl2,
            tc.tile_pool(name="pl3", bufs=1, space="PSUM") as pl3,
            tc.tile_pool(name="pP", bufs=1, space="PSUM") as pP_pool,
            tc.tile_pool(name="plog", bufs=1, space="PSUM") as plog_pool,
        ):
            loop_cm = tc.For_i(0, loop) if loop is not None else nullcontext()
            with loop_cm:
                # x arrives pre-transposed (features on rows): load the 8
                # k-tiles [128, BPC] directly — no on-chip transpose needed.
                xt = []
                for f in range(KF):
                    t = xt_pool.tile([128, BPC], bf16, tag="xt",
                                     name=f"xt{f}")
                    # Pool/Act queues (SP is busy with the consts DMAs) so
                    # the x fill streams in f-order alongside the L1 matmuls
                    eng = nc.gpsimd if f % 2 == 0 else nc.scalar
                    eng.dma_start(out=t, in_=x_d[f * 128:(f + 1) * 128, :])
                    xt.append(t)

                # L1: d1 = relu(x @ W1aug + b1aug); one stationary load per
                # (m, f), reused across all 4 batch tiles.  d1[n][m] tiles.
                d1 = [[None] * 4 for _ in range(NT)]
                for m in range(4):
                    p1s = [pl1.tile([128, NTILE], f32, tag="p1",
                                    name=f"p1_{m}_{n}") for n in range(NT)]
                    for f in range(KF):
                        for n in range(NT):
                            nc.tensor.matmul(
                                p1s[n], w1sb[f][:, m * 128:(m + 1) * 128],
                                xt[f][:, n * NTILE:(n + 1) * NTILE],
                                start=(f == 0), stop=(f == KF - 1))
                    for n in range(NT):
                        dt_ = d1_pool.tile([128, NTILE], bf16, tag="d1",
                                           name=f"d1_{m}_{n}")
                        nc.scalar.activation(out=dt_, in_=p1s[n], func=AF.Relu,
                                             bias=b1sb[:, m:m + 1], scale=1.0)
                        if m == 3:
                            # overwrite tail rows with raw psum + c-constants
                            nc.vector.tensor_scalar_add(
                                dt_[96:104, :], p1s[n][96:104, :],
                                b1sb[96:104, 3:4])
                        d1[n][m] = dt_

                for n in range(NT):
                    base = n * NTILE
                    d1n = d1[n]
                    d13 = d1n[3]

                    # tail products: three rounds of T *= (mask_one + mask_s*s)
                    # on a PRIVATE tile T (copy-free: round 0 writes T from
                    # d13's rows) so L2/L3 never wait on the cubic chain.
                    # tail rows 96..103 of d13 =
                    #   [s, y1_0, y2_0, y3_0, y1_1, y2_1, y3_1, y0]
                    # psS rows 0:8 = s broadcast (K=1 matmul, proven on HW;
                    # the DVE chain below sits at partitions 96:104 — the HW
                    # verifier requires TensorTensor operands to share the
                    # start partition, and TensorScalar allows the shift).
                    psS = pP_pool.tile([128, NTILE], f32, tag="pP",
                                       name=f"psS{n}")
                    nc.tensor.matmul(psS[0:8, :], ones8[96:97, :],
                                     d13[96:97, :], start=True, stop=True,
                                     tile_position=(96, 0))
                    T = ss_pool.tile([128, NTILE], bf16, tag="Ttail",
                                     name=f"T{n}")

                    def tail_round(j):
                        tmp = tmp_pool.tile([128, NTILE], f32, tag="tmp",
                                            name=f"tmp{n}_{j}")
                        nc.vector.tensor_scalar(
                            out=tmp[96:104, :], in0=psS[0:8, :],
                            scalar1=maskbuf[96:104, j:j + 1],
                            scalar2=maskbuf[96:104, 3 + j:4 + j],
                            op0=mybir.AluOpType.mult, op1=mybir.AluOpType.add)
                        src = d13[96:104, :] if j == 0 else T[96:104, :]
                        nc.vector.tensor_mul(T[96:104, :], src,
                                             tmp[96:104, :])

                    tail_round(0)

                    # L2: d2 = relu(d1 @ W2aug + b2)  (tail rows hit 0 weights)
                    d2 = []
                    for m in range(2):
                        p2 = pl2.tile([100, NTILE], f32, tag="p2",
                                      name=f"p2_{n}_{m}")
                        for k in range(4):
                            nc.tensor.matmul(
                                p2, w2sb[k][:, m * 100:(m + 1) * 100], d1n[k],
                                start=(k == 0), stop=(k == 3))
                        t2 = d2_pool.tile([100, NTILE], bf16, tag="d2",
                                          name=f"d2_{n}_{m}")
                        nc.scalar.activation(out=t2, in_=p2, func=AF.Relu,
                                             bias=b2sb[:, m:m + 1], scale=1.0)
                        d2.append(t2)

                    tail_round(1)

                    # L3: d3 = relu(d2 @ W3 + b3)
                    d3 = []
                    for m in range(2):
                        p3 = pl3.tile([100, NTILE], f32, tag="p3",
                                      name=f"p3_{n}_{m}")
                        for k in range(2):
                            nc.tensor.matmul(
                                p3, w3sb[k][:, m * 100:(m + 1) * 100], d2[k],
                                start=(k == 0), stop=(k == 1))
                        t3 = d3_pool.tile([100, NTILE], bf16, tag="d3",
                                          name=f"d3_{n}_{m}")
                        nc.scalar.activation(out=t3, in_=p3, func=AF.Relu,
                                             bias=b3sb[:, m:m + 1], scale=1.0)
                        d3.append(t3)

                    tail_round(2)

                    # logits: selection matmul over tail rows + d3 @ Wl_d3
                    pl = plog_pool.tile([2, NTILE], f32, tag="plg",
                                        name=f"plog{n}")
                    nc.tensor.matmul(pl, selsb[96:104, :], T[96:104, :],
                                     start=True, stop=False,
                                     tile_position=(96, 0))
                    nc.tensor.matmul(pl, wd3sb[0], d3[0],
                                     start=False, stop=False)
                    nc.tensor.matmul(pl, wd3sb[1], d3[1],
                                     start=False, stop=True)

                    osb = out_pool.tile([2, NTILE], f32, tag="osb",
                                        name=f"osb{n}")
                    nc.scalar.activation(out=osb, in_=pl, func=AF.Sigmoid,
                                         bias=sigbsb, scale=1.0)
                    nc.sync.dma_start(out=out_d[:, base:base + NTILE], in_=osb)

    nc.finalize()
    return nc


def _prep_host(W1, b1, W2, b2, W3, b3, Wl, bl, cw, cb):
    """Build the augmented/permuted parameter arrays (bf16 weights)."""
    bf16 = _np_bf16()
    W1 = np.asarray(W1, np.float32)
    b1 = np.asarray(b1, np.float32)
    W2 = np.asarray(W2, np.float32)
    b2 = np.asarray(b2, np.float32)
    W3 = np.asarray(W3, np.float32)
    b3 = np.asarray(b3, np.float32)
    Wl = np.asarray(Wl, np.float32)
    bl = np.asarray(bl, np.float32)
    cw = np.asarray(cw, np.float32)
    cb = np.asarray(cb, np.float32)

    w_emb = Wl[:DIM, 0]
    w_d3 = Wl[DIM:, 0]

    u = np.zeros((DIM, 8), np.float32)
    u[:, 0] = 1.0                      # s = x @ ones
    c1 = np.zeros(2, np.float32)
    c2 = np.zeros(2, np.float32)
    c0 = np.zeros(2, np.float32)
    for i in range(2):
        cw2 = cw[i, 2]
        cw12 = cw[i, 1] * cw2
        cw012 = cw[i, 0] * cw12
        u[:, 1 + 3 * i] = cw2 * w_emb
        u[:, 2 + 3 * i] = cw12 * w_emb
        u[:, 3 + 3 * i] = cw012 * w_emb
        c1[i] = float(np.dot(cb[i, 1] * cw2, w_emb))
        c2[i] = float(np.dot(cb[i, 0] * cw12, w_emb))
        c0[i] = float(np.dot(cb[i, 2], w_emb))
    u[:, 7] = w_emb                    # y0 = x @ w_emb

    w1aug = np.zeros((DIM, 512), np.float32)
    w1aug[:, 0:480] = W1[:, 0:480]
    w1aug[:, 480:488] = u
    w1aug[:, 488:508] = W1[:, 480:500]

    b1full = np.zeros(512, np.float32)
    b1full[0:480] = b1[0:480]
    b1full[480:488] = [0.0, c1[0], c2[0], 0.0, c1[1], c2[1], 0.0, 0.0]
    b1full[488:508] = b1[480:500]
    b1aug = np.ascontiguousarray(b1full.reshape(4, 128).T)

    w2aug = np.zeros((512, H2), np.float32)
    w2aug[0:480] = W2[0:480]
    w2aug[488:508] = W2[480:500]

    sel = np.zeros((128, 2), np.float32)
    sel[97:100, 0] = 1.0
    sel[103, 0] = 1.0
    sel[100:103, 1] = 1.0
    sel[103, 1] = 1.0

    wd3dup = np.ascontiguousarray(np.stack([w_d3, w_d3], axis=1))
    b2arr = np.ascontiguousarray(b2.reshape(2, 100).T)
    b3arr = np.ascontiguousarray(b3.reshape(2, 100).T)
    sigb = np.array([[c0[0] + bl[0]], [c0[1] + bl[0]]], np.float32)

    # tail-round masks: round j multiplies tail row r by
    # (mask_one[j][r] + mask_s[j][r]*s); after 3 rounds the rows
    # [s, y1_0, y2_0, y3_0, y1_1, y2_1, y3_1, y0] carry [s, y1*s, y2*s^2,
    # y3*s^3, ..., y0].  tailmask[:, j] = mask_s, tailmask[:, 3+j] = mask_one.
    tailmask = np.zeros((128, 6), np.float32)
    ones_masks = [[1, 0, 0, 0, 0, 0, 0, 1],
                  [1, 1, 0, 0, 1, 0, 0, 1],
                  [1, 1, 1, 0, 1, 1, 0, 1]]
    s_masks = [[0, 1, 1, 1, 1, 1, 1, 0],
               [0, 0, 1, 1, 0, 1, 1, 0],
               [0, 0, 0, 1, 0, 0, 1, 0]]
    for j in range(3):
        tailmask[96:104, j] = s_masks[j]
        tailmask[96:104, 3 + j] = ones_masks[j]

    return dict(w1aug=w1aug.astype(bf16), w2aug=w2aug.astype(bf16),
                w3m=np.ascontiguousarray(W3).astype(bf16),
                wd3dup=wd3dup.astype(bf16), sel=sel.astype(bf16),
                b1aug=b1aug, b2arr=b2arr, b3arr=b3arr, sigb=sigb,
                tailmask=tailmask, onesrow=np.ones((1, 8), bf16))


def _make_exec(nc):
    """Build a jitted shard_map executor for `nc` over the 8-core mesh.

    Returns (fn, in_names, out_names, out_shapes).  fn takes
    (dict name->global ndarray for sharded inputs, dict name->ndarray for
    replicated inputs) and returns the global output arrays.  The x shard
    input is passed as one (B, DIM) array split on axis 0 by the mesh;
    everything else is replicated.  Output buffers are pre-zeroed and
    donated (the custom call writes into them).
    """
    import jax
    import numpy as jnp_np
    from jax.sharding import Mesh, PartitionSpec as P
    from jax.experimental.shard_map import shard_map
    import concourse.mybir as mybir
    from concourse.bass2jax import (
        install_neuronx_cc_hook, _bass_exec_p, partition_id_tensor)

    install_neuronx_cc_hook()
    assert not nc.dbg_callbacks
    part_name = (nc.partition_id_tensor.name
                 if nc.partition_id_tensor is not None else None)

    in_names, out_names, out_avals, zero_outs = [], [], [], []
    for alloc in nc.m.functions[0].allocations:
        if not isinstance(alloc, mybir.MemoryLocationSet):
            continue
        name = alloc.memorylocations[0].name
        if alloc.kind == "ExternalInput":
            if name != part_name:
                in_names.append(name)
        elif alloc.kind == "ExternalOutput":
            shape = tuple(alloc.tensor_shape)
            dtype = mybir.dt.np(alloc.dtype)
            out_names.append(name)
            out_avals.append(jax.core.ShapedArray(shape, dtype))
            zero_outs.append(np.zeros((NCORES * shape[0],) + shape[1:], dtype))
    n_params = len(in_names)
    all_names = in_names + out_names
    if part_name is not None:
        all_names = all_names + [part_name]

    def _body(*args):
        operands = list(args)
        if part_name is not None:
            operands.append(partition_id_tensor())
        outs = _bass_exec_p.bind(
            *operands,
            out_avals=tuple(out_avals),
            in_names=tuple(all_names),
            out_names=tuple(out_names),
            lowering_input_output_aliases=(),
            sim_require_finite=True,
            sim_require_nnan=True,
            nc=nc,
        )
        return tuple(outs)

    devices = jax.devices()[:NCORES]
    mesh = Mesh(jnp_np.asarray(devices), ("core",))
    in_specs = tuple(
        P(None, "core") if name == "xT_shard" else P() for name in in_names
    ) + (P("core"),) * len(out_names)
    out_specs = (P("core"),) * len(out_names)
    donate = tuple(range(n_params, n_params + len(out_names)))
    fn = jax.jit(
        shard_map(_body, mesh=mesh, in_specs=in_specs, out_specs=out_specs,
                  check_rep=False),
        donate_argnums=donate, keep_unused=True)
    return fn, in_names, out_names, zero_outs


def _get_exec(key, loop=None):
    if key not in _CACHE:
        nc = _build_nc(loop=loop)
        _CACHE[key] = _make_exec(nc)
    return _CACHE[key]


def _run(fn, in_names, zero_outs, in_map):
    args = [in_map.get(name, np.zeros((1, 2), np.uint32))
            for name in in_names]
    args += [np.copy(z) for z in zero_outs]
    return fn(*args)


def kernel(x, show_index, st, W1, b1, W2, b2, W3, b3, Wl, bl, cw, cb):
    import jax

    bf16 = _np_bf16()
    # transpose + cast x on the CPU backend (vectorized, faster than numpy)
    cpu = jax.devices("cpu")[0]
    with jax.default_device(cpu):
        x_bf = np.asarray(
            jax.jit(lambda a: a.T.astype(bf16))(np.asarray(x)))

    in_map = {"xT_shard": x_bf}
    in_map.update(_prep_host(W1, b1, W2, b2, W3, b3, Wl, bl, cw, cb))

    fn, in_names, out_names, zero_outs = _get_exec("full")
    outs = _run(fn, in_names, zero_outs, in_map)
    preds = np.asarray(outs[out_names.index("preds")])  # (16, BPC)
    # core c contributed rows [2c, 2c+1]: row 2c = task0, 2c+1 = task1
    p0 = np.ascontiguousarray(preds[0::2].reshape(B, 1)).astype(np.float32)
    p1 = np.ascontiguousarray(preds[1::2].reshape(B, 1)).astype(np.float32)
    return (p0, p1)
